# revision 7
# baseline (speedup 1.0000x reference)
"""GraphSAGE (gnn_message_passing) forward pass on 8 Trainium2 NeuronCores.

Sharding strategy (hardcoded): row-shard the 10000 nodes across 8 cores
(1250 each, padded to 1280).  Each core receives its shard of the adjacency
matrix pre-transposed ([10240, 1280] bf16, columns = this core's nodes) so
the aggregation matmuls stream contiguous natural-layout slabs.  Node
features are kept feature-major ([feat_part, node_free]) on-chip so every
linear layer is a natural matmul.  Between GNN layers the updated node
features are AllGathered (bf16, node-major) so every core sees all nodes
for the next aggregation.  Small weights / LSTM params are replicated.
"""

import os
from contextlib import ExitStack

import numpy as np
import ml_dtypes

import concourse.bass as bass
import concourse.bacc as bacc
import concourse.mybir as mybir
import concourse.tile as tile
from concourse.bass_utils import run_bass_kernel_spmd
from concourse.masks import make_identity

F32 = mybir.dt.float32
BF16 = mybir.dt.bfloat16
AX = mybir.AxisListType
OP = mybir.AluOpType
AF = mybir.ActivationFunctionType

# ---- problem constants (hardcoded per spec) ----
N = 10000        # nodes
NC = 8           # cores
NPC = 1250       # original nodes per core
PC = 1280        # padded nodes per core
NP = NC * PC     # padded total nodes = 10240
KT = NP // 128   # 80 contraction tiles
IT = PC // 128   # 10 node tiles per core
NFEAT = 2000
FPAD = 2048
FT = FPAD // 128  # 16
NH = 128
NHE = 64
NFE = 256
D = NH + NHE     # 192
NOUT = 20
L = 2
BN_EPS = 1e-5

# matmul free-dim chunks over PC (PSUM bank = 512 fp32)
CHUNKS = [(0, 512), (512, 512), (1024, 256)]

LAST_RESULT = None  # test.py reads exec_time info from here

_CACHED_NC = None


def _bf(a):
    return np.asarray(a, dtype=ml_dtypes.bfloat16)


def _f32(a):
    return np.ascontiguousarray(a, dtype=np.float32)


# --------------------------------------------------------------------------
# device program
# --------------------------------------------------------------------------

def _build_program():
    nc = bacc.Bacc("TRN2", target_bir_lowering=False, debug=False, num_devices=NC)

    def inp(name, shape, dtype):
        return nc.declare_dram_parameter(name, list(shape), dtype, isOutput=False)

    # per-core tensors
    d_adjT = inp("adjT", [NP, PC], BF16)
    d_xT = inp("xT", [FPAD, PC], BF16)
    d_embT = inp("embT", [NFE, PC], F32)
    d_rsb = inp("rsb", [128, PC], F32)          # 1/rowsum broadcast to 128 parts
    # replicated weights
    d_w_inT = inp("w_inT", [FPAD, NH], BF16)
    d_sc_in = inp("sc_in", [NH, 1], F32)
    d_sh_in = inp("sh_in", [NH, 1], F32)        # with b_in folded
    d_sc_in_h = inp("sc_in_h", [NH, 1], F32)    # 0.5 * sc_in (for JK mean)
    d_sh_in2 = inp("sh_in2", [NH, 1], F32)      # without linear bias
    d_wgs_sT = inp("wgs_sT", [L, NH, NH], F32)
    d_wgs_nT = inp("wgs_nT", [L, NH, NH], F32)
    d_bgs = inp("bgs", [NH, L], F32)
    d_wihT = inp("wihT", [L, NH, 4 * NH], BF16)
    d_whhT = inp("whhT", [L, NH, 4 * NH], BF16)
    d_blstm = inp("blstm", [NH, 2 * 4], F32)    # (l, gate) on free dim
    d_w_embT = inp("w_embT", [NFE, NHE], F32)
    d_sc_emb = inp("sc_emb", [NHE, 1], F32)
    d_sh_emb = inp("sh_emb", [NHE, 1], F32)
    d_w_fcT = inp("w_fcT", [D, D], F32)
    d_sc_fc_a = inp("sc_fc_a", [128, 1], F32)
    d_sh_fc_a = inp("sh_fc_a", [128, 1], F32)
    d_sc_fc_b = inp("sc_fc_b", [64, 1], F32)
    d_sh_fc_b = inp("sh_fc_b", [64, 1], F32)
    d_w_outT = inp("w_outT", [D, NOUT], F32)
    d_bout = inp("bout", [128, NOUT], F32)      # b_out broadcast across parts
    d_out = nc.declare_dram_parameter("out", [PC, NOUT], F32, isOutput=True)

    # internal DRAM for collectives
    bounce = [nc.dram_tensor(f"bounce{l}", [IT, 128, 128], BF16) for l in range(L)]
    hg = [
        nc.dram_tensor(f"hg{l}", [KT, 128, 128], BF16, addr_space="Shared")
        for l in range(L)
    ]
    groups = [list(range(NC))]

    with tile.TileContext(nc) as tc, ExitStack() as top:
        const = top.enter_context(tc.tile_pool(name="const", bufs=1))
        persist = top.enter_context(tc.tile_pool(name="persist", bufs=1))
        tmpf = top.enter_context(tc.tile_pool(name="tmpf", bufs=1))
        slabp = top.enter_context(tc.tile_pool(name="slab", bufs=4))
        hnatp = top.enter_context(tc.tile_pool(name="hnat", bufs=1))

        # ---- load constants ----
        w_in = const.tile([128, FPAD], BF16, tag="w_in")
        nc.sync.dma_start(w_in.rearrange("p (t j) -> p t j", t=FT),
                          d_w_inT.ap().rearrange("(t p) j -> p t j", p=128))
        wgs_s = [const.tile([128, NH], F32, tag=f"wgs_s{l}", name=f"wgs_s{l}")
                 for l in range(L)]
        wgs_n = [const.tile([128, NH], F32, tag=f"wgs_n{l}", name=f"wgs_n{l}")
                 for l in range(L)]
        for l in range(L):
            nc.sync.dma_start(wgs_s[l], d_wgs_sT[l])
            nc.sync.dma_start(wgs_n[l], d_wgs_nT[l])
        bgs = const.tile([128, L], F32, tag="bgs")
        nc.sync.dma_start(bgs, d_bgs.ap())
        wih = [const.tile([128, 4 * NH], BF16, tag=f"wih{l}", name=f"wih{l}")
               for l in range(L)]
        whh = [const.tile([128, 4 * NH], BF16, tag=f"whh{l}", name=f"whh{l}")
               for l in range(L)]
        for l in range(L):
            nc.sync.dma_start(wih[l], d_wihT[l])
            nc.sync.dma_start(whh[l], d_whhT[l])
        blstm = const.tile([128, 8], F32, tag="blstm")
        nc.sync.dma_start(blstm, d_blstm.ap())
        w_emb = [const.tile([128, NHE], F32, tag=f"w_emb{t}", name=f"w_emb{t}")
                 for t in range(2)]
        for t in range(2):
            nc.sync.dma_start(w_emb[t], d_w_embT[t * 128:(t + 1) * 128, :])
        wfc_aa = const.tile([128, 128], F32, tag="wfc_aa")
        wfc_ba = const.tile([64, 128], F32, tag="wfc_ba")
        wfc_ab = const.tile([128, 64], F32, tag="wfc_ab")
        wfc_bb = const.tile([64, 64], F32, tag="wfc_bb")
        nc.sync.dma_start(wfc_aa, d_w_fcT[:128, :128])
        nc.sync.dma_start(wfc_ba, d_w_fcT[128:, :128])
        nc.sync.dma_start(wfc_ab, d_w_fcT[:128, 128:])
        nc.sync.dma_start(wfc_bb, d_w_fcT[128:, 128:])
        w_out_a = const.tile([128, NOUT], F32, tag="w_out_a")
        w_out_b = const.tile([64, NOUT], F32, tag="w_out_b")
        nc.sync.dma_start(w_out_a, d_w_outT[:128, :])
        nc.sync.dma_start(w_out_b, d_w_outT[128:, :])
        bout = const.tile([128, NOUT], F32, tag="bout")
        nc.sync.dma_start(bout, d_bout.ap())
        rsb = const.tile([128, PC], F32, tag="rsb")
        nc.sync.dma_start(rsb, d_rsb.ap())

        small = {}
        for nm, dd, p in [
            ("sc_in", d_sc_in, NH), ("sh_in", d_sh_in, NH),
            ("sc_in_h", d_sc_in_h, NH), ("sh_in2", d_sh_in2, NH),
            ("sc_emb", d_sc_emb, NHE), ("sh_emb", d_sh_emb, NHE),
            ("sc_fc_a", d_sc_fc_a, 128), ("sh_fc_a", d_sh_fc_a, 128),
            ("sc_fc_b", d_sc_fc_b, 64), ("sh_fc_b", d_sh_fc_b, 64),
        ]:
            t = const.tile([p, 1], F32, tag=nm, name=nm)
            nc.sync.dma_start(t, dd.ap())
            small[nm] = t

        ident_bf = const.tile([128, 128], BF16, tag="ident")
        make_identity(nc, ident_bf)
        ones_col = const.tile([128, 1], F32, tag="ones_col")
        nc.vector.memset(ones_col, 1.0)
        ones_row = const.tile([1, 128], F32, tag="ones_row")
        nc.vector.memset(ones_row, 1.0)

        # ---- helpers ----
        def mm_acc(psum_ap, lhsT, rhs, start, stop):
            """accumulate lhsT.T @ rhs into psum, chunking free dim at 512"""
            F = rhs.shape[-1]
            o = 0
            while o < F:
                w = min(512, F - o)
                nc.tensor.matmul(
                    psum_ap[:, o:o + w], lhsT, rhs[:, o:o + w],
                    start=start, stop=stop,
                )
                o += w

        def elu_from(out_sb, in_ap, sc_ap, sh_ap):
            """out = elu(sc*in + sh); in_ap may be PSUM; [P, F]"""
            P, F = out_sb.shape[0], out_sb.shape[-1]
            y = tmpf.tile([128, PC], F32, tag="elu_y", name="elu_y")[:P, :F]
            nc.vector.tensor_scalar(y, in_ap, sc_ap, sh_ap, OP.mult, OP.add)
            e = tmpf.tile([128, PC], F32, tag="elu_e", name="elu_e")[:P, :F]
            nc.vector.tensor_scalar_min(e, y, 0.0)
            nc.scalar.activation(e, e, AF.Exp)
            # y <- max(y,0) - 1   (in place)
            nc.vector.tensor_scalar(y, y, 0.0, -1.0, OP.max, OP.add)
            nc.vector.tensor_tensor(out_sb, y, e, OP.add)

        # persistent activations
        hT = [persist.tile([128, PC], F32, tag="hT", bufs=2, name=f"hT{l}")
              for l in range(3)]
        hT_bf = [persist.tile([128, PC], BF16, tag=f"hTbf{l}", name=f"hTbf{l}")
                 for l in range(3)]

        with tc.tile_pool(name="psA", bufs=1, space="PSUM") as psA, \
             tc.tile_pool(name="psS", bufs=1, space="PSUM") as psS, \
             tc.tile_pool(name="psB", bufs=1, space="PSUM") as psB, \
             tc.tile_pool(name="psT", bufs=2, space="PSUM") as psT, \
             tc.tile_pool(name="tmpc", bufs=2) as tmpc:

            def gather(l, src_bf):
                """transpose local hT bf16 to node-major, AllGather into hg[l]"""
                loc = tmpc.tile([128, IT * 128], BF16, tag="hnat_loc",
                                name="hnat_loc")
                for it in range(IT):
                    pt = psT.tile([128, 128], BF16, tag="tp", name="tp")
                    nc.tensor.transpose(
                        pt, src_bf[:, it * 128:(it + 1) * 128], ident_bf)
                    nc.vector.tensor_copy(loc[:, it * 128:(it + 1) * 128], pt)
                nc.sync.dma_start(
                    bounce[l].ap().rearrange("t p f -> p t f"),
                    loc.rearrange("p (t f) -> p t f", t=IT))
                nc.gpsimd.collective_compute(
                    "AllGather", OP.bypass, replica_groups=groups,
                    ins=[bounce[l].ap().opt()], outs=[hg[l].ap().opt()],
                )

            # ---- input projection: h0T = elu(bn(W_in @ x)) ----
            ps = psA.tile([128, PC], F32, tag="big", name="ps_proj")
            for t in range(FT):
                xt = tmpf.tile([128, PC], BF16, tag="xstream", bufs=3,
                               name="xt")
                nc.sync.dma_start(xt, d_xT[t * 128:(t + 1) * 128, :])
                mm_acc(ps, w_in[:, t * 128:(t + 1) * 128], xt,
                       start=(t == 0), stop=(t == FT - 1))
            elu_from(hT[0], ps, small["sc_in"], small["sh_in"])
            nc.vector.tensor_copy(hT_bf[0], hT[0])
            gather(0, hT_bf[0])

            # ---- GNN layers ----
            for l in range(L):
                hnat = hnatp.tile([128, KT * 128], BF16, tag="hnat",
                                  name="hnat")
                nc.sync.dma_start(hnat.rearrange("p (t f) -> p t f", t=KT),
                                  hg[l].ap().rearrange("t p f -> p t f"))
                ps_agg = psA.tile([128, PC], F32, tag="big", name="ps_agg")
                for kt in range(KT):
                    slab = slabp.tile([128, PC], BF16, tag="slab", name="slab")
                    nc.sync.dma_start(slab, d_adjT[kt * 128:(kt + 1) * 128, :])
                    mm_acc(ps_agg, hnat[:, kt * 128:(kt + 1) * 128], slab,
                           start=(kt == 0), stop=(kt == KT - 1))
                neighT = tmpf.tile([128, PC], F32, tag="neighT", name="neighT")
                nc.vector.tensor_tensor(neighT, ps_agg, rsb, OP.mult)

                # GS linear: relu(W_self @ h + W_neigh @ neigh + b)
                ps_gs = psA.tile([128, PC], F32, tag="big", name="ps_gs")
                mm_acc(ps_gs, wgs_s[l], hT[l], start=True, stop=False)
                mm_acc(ps_gs, wgs_n[l], neighT, start=False, stop=True)
                hrelu = tmpf.tile([128, PC], F32, tag="hrelu", name="hrelu")
                nc.scalar.activation(hrelu, ps_gs, AF.Relu,
                                     bias=bgs[:, l:l + 1], scale=1.0)

                # L2 normalize along features (partition dim) via PE ones-reduce
                sq = tmpf.tile([128, PC], F32, tag="sq", name="sq")
                nc.vector.tensor_tensor(sq, hrelu, hrelu, OP.mult)
                nrm = tmpf.tile([1, PC], F32, tag="nrm", name="nrm")
                for (o, w) in CHUNKS:
                    ps_ss = psS.tile([1, 512], F32, tag="ss", name="ps_ss")
                    nc.tensor.matmul(ps_ss[:, :w], ones_col, sq[:, o:o + w],
                                     start=True, stop=True)
                    nc.scalar.activation(nrm[:, o:o + w], ps_ss[:, :w], AF.Sqrt)
                nc.vector.tensor_scalar_max(nrm, nrm, 1e-12)
                rec = tmpf.tile([1, PC], F32, tag="rec", name="rec")
                nc.vector.reciprocal(rec, nrm)
                for (o, w) in CHUNKS:
                    ps_bc = psB.tile([128, 512], F32, tag="bc", name="ps_bc")
                    nc.tensor.matmul(ps_bc[:, :w], ones_row, rec[:, o:o + w],
                                     start=True, stop=True)
                    nc.vector.tensor_tensor(hT[l + 1][:, o:o + w],
                                            hrelu[:, o:o + w], ps_bc[:, :w],
                                            OP.mult)
                nc.vector.tensor_copy(hT_bf[l + 1], hT[l + 1])
                if l == 0:
                    gather(1, hT_bf[1])

        # ---- 2-layer LSTM jumping knowledge over T=2 ----
        c_st = [persist.tile([128, PC], F32, tag=f"c{l}", name=f"c{l}")
                for l in range(2)]
        o_bf = [persist.tile([128, PC], BF16, tag=f"o{t}", name=f"o{t}")
                for t in range(2)]
        p0_bf = persist.tile([128, PC], BF16, tag="p0bf")
        p_f = [persist.tile([128, PC], F32, tag=f"p{t}f", name=f"p{t}f")
               for t in range(2)]

        with tc.tile_pool(name="psL", bufs=2, space="PSUM") as psL, \
             tc.tile_pool(name="tmpg", bufs=1) as tmpg:

            def lstm_cell(l, t, xin_bf, hprev_bf, c_tile, out_f32, out_bf):
                for (o, w) in CHUNKS:
                    gps = [psL.tile([128, 512], F32, tag=f"g{g}", name=f"g{g}")
                           for g in range(4)]
                    for g in range(4):
                        nc.tensor.matmul(
                            gps[g][:, :w],
                            wih[l][:, g * 128:(g + 1) * 128],
                            xin_bf[:, o:o + w],
                            start=True, stop=(t == 0))
                        if t > 0:
                            nc.tensor.matmul(
                                gps[g][:, :w],
                                whh[l][:, g * 128:(g + 1) * 128],
                                hprev_bf[:, o:o + w],
                                start=False, stop=True)
                    gact = []
                    for g, fn in enumerate([AF.Sigmoid, AF.Sigmoid,
                                            AF.Tanh, AF.Sigmoid]):
                        gt = tmpg.tile([128, 512], F32, tag=f"ga{g}",
                                       name=f"ga{g}")[:, :w]
                        nc.scalar.activation(gt, gps[g][:, :w], fn,
                                             bias=blstm[:, l * 4 + g:l * 4 + g + 1])
                        gact.append(gt)
                    ig, fg, gg, og = gact
                    cs = c_tile[:, o:o + w]
                    if t == 0:
                        nc.vector.tensor_tensor(cs, ig, gg, OP.mult)
                    else:
                        fc_ = tmpg.tile([128, 512], F32, tag="fc_",
                                        name="fc_")[:, :w]
                        nc.vector.tensor_tensor(fc_, fg, cs, OP.mult)
                        igg = tmpg.tile([128, 512], F32, tag="igg",
                                        name="igg")[:, :w]
                        nc.vector.tensor_tensor(igg, ig, gg, OP.mult)
                        nc.vector.tensor_tensor(cs, fc_, igg, OP.add)
                    tc_ = tmpg.tile([128, 512], F32, tag="tc_",
                                    name="tc_")[:, :w]
                    nc.scalar.activation(tc_, cs, AF.Tanh)
                    if out_f32 is not None:
                        nc.vector.tensor_tensor(out_f32[:, o:o + w], og, tc_,
                                                OP.mult)
                        if out_bf is not None:
                            nc.vector.tensor_copy(out_bf[:, o:o + w],
                                                  out_f32[:, o:o + w])
                    else:
                        nc.vector.tensor_tensor(out_bf[:, o:o + w], og, tc_,
                                                OP.mult)

            # layer0 t0; layer1 t0; layer0 t1; layer1 t1
            lstm_cell(0, 0, hT_bf[1], None, c_st[0], None, o_bf[0])
            lstm_cell(1, 0, o_bf[0], None, c_st[1], p_f[0], p0_bf)
            lstm_cell(0, 1, hT_bf[2], o_bf[0], c_st[0], None, o_bf[1])
            lstm_cell(1, 1, o_bf[1], p0_bf, c_st[1], p_f[1], None)

        # ---- post: JK mean -> bn/elu ; embed ; fc ; logits ; log_softmax ----
        hpost = persist.tile([128, PC], F32, tag="hpost")
        eT = persist.tile([64, PC], F32, tag="eT")
        hfca = persist.tile([128, PC], F32, tag="hfca")
        hfcb = persist.tile([64, PC], F32, tag="hfcb")
        outall = persist.tile([128, IT * NOUT], F32, tag="outall")

        with tc.tile_pool(name="psP", bufs=2, space="PSUM") as psP, \
             tc.tile_pool(name="psG", bufs=2, space="PSUM") as psG, \
             tc.tile_pool(name="tmps", bufs=2) as tmps:

            hsum = tmpf.tile([128, PC], F32, tag="neighT", name="hsum")
            nc.vector.tensor_tensor(hsum, p_f[0], p_f[1], OP.add)
            # 0.5 from the mean is folded into sc_in_h
            elu_from(hpost, hsum, small["sc_in_h"], small["sh_in2"])

            # embed projection
            ps_e = psP.tile([128, PC], F32, tag="post", name="ps_e")
            for t in range(2):
                et = tmpf.tile([128, PC], F32, tag="sq", name="et")
                nc.sync.dma_start(et, d_embT[t * 128:(t + 1) * 128, :])
                mm_acc(ps_e[:64, :], w_emb[t], et, start=(t == 0), stop=(t == 1))
            elu_from(eT, ps_e[:64, :], small["sc_emb"], small["sh_emb"])

            # fc on concat([hpost, eT]) without materializing the concat
            ps_fa = psP.tile([128, PC], F32, tag="post", name="ps_fa")
            mm_acc(ps_fa, wfc_aa, hpost, start=True, stop=False)
            mm_acc(ps_fa, wfc_ba, eT, start=False, stop=True)
            elu_from(hfca, ps_fa, small["sc_fc_a"], small["sh_fc_a"])
            ps_fb = psP.tile([128, PC], F32, tag="post", name="ps_fb")
            mm_acc(ps_fb[:64, :], wfc_ab, hpost, start=True, stop=False)
            mm_acc(ps_fb[:64, :], wfc_bb, eT, start=False, stop=True)
            elu_from(hfcb, ps_fb[:64, :], small["sc_fc_b"], small["sh_fc_b"])

            # logits per node-tile (natural orientation) + log_softmax
            for it in range(IT):
                ps_lg = psG.tile([128, NOUT], F32, tag="lg", name="ps_lg")
                nc.tensor.matmul(ps_lg, hfca[:, it * 128:(it + 1) * 128],
                                 w_out_a, start=True, stop=False)
                nc.tensor.matmul(ps_lg, hfcb[:, it * 128:(it + 1) * 128],
                                 w_out_b, start=False, stop=True)
                lg = tmps.tile([128, NOUT], F32, tag="lg_sb", name="lg_sb")
                nc.vector.tensor_tensor(lg, ps_lg, bout, OP.add)
                mx = tmps.tile([128, 1], F32, tag="mx", name="mx")
                nc.vector.tensor_reduce(mx, lg, AX.X, OP.max)
                sh = tmps.tile([128, NOUT], F32, tag="shift", name="shifted")
                nc.vector.tensor_scalar(sh, lg, mx, None, OP.subtract)
                ex = tmps.tile([128, NOUT], F32, tag="ex", name="ex")
                se = tmps.tile([128, 1], F32, tag="se", name="se")
                nc.scalar.activation(ex, sh, AF.Exp, accum_out=se)
                lse = tmps.tile([128, 1], F32, tag="lse", name="lse")
                nc.scalar.activation(lse, se, AF.Ln)
                nc.vector.tensor_scalar(
                    outall[:, it * NOUT:(it + 1) * NOUT], sh, lse, None,
                    OP.subtract)

            nc.sync.dma_start(
                d_out.ap().rearrange("(t p) c -> p t c", p=128),
                outall.rearrange("p (t c) -> p t c", t=IT))

    nc.compile()
    return nc


# --------------------------------------------------------------------------
# host side
# --------------------------------------------------------------------------

def _stage_inputs(
    x, embed, adj, W_in, b_in, bn_in_g, bn_in_b, bn_in_rm, bn_in_rv,
    W_gs, b_gs, Wih0, Whh0, bih0, bhh0, Wih1, Whh1, bih1, bhh1,
    W_emb, b_emb, bn_emb_g, bn_emb_b, bn_emb_rm, bn_emb_rv,
    W_fc, b_fc, bn_fc_g, bn_fc_b, bn_fc_rm, bn_fc_rv, W_out, b_out,
):
    x = np.asarray(x, np.float32)
    embed = np.asarray(embed, np.float32)
    adj = np.asarray(adj, np.float32)

    # replicated weight staging
    w_inT = np.zeros((FPAD, NH), ml_dtypes.bfloat16)
    w_inT[:NFEAT] = _bf(np.asarray(W_in, np.float32).T)

    def bn_fold(g, b, rm, rv, lin_b=None):
        g = np.asarray(g, np.float32); b = np.asarray(b, np.float32)
        rm = np.asarray(rm, np.float32); rv = np.asarray(rv, np.float32)
        sc = g / np.sqrt(rv + BN_EPS)
        base = lin_b if lin_b is not None else 0.0
        shv = sc * (base - rm) + b
        return _f32(sc), _f32(shv)

    sc_in, sh_in = bn_fold(bn_in_g, bn_in_b, bn_in_rm, bn_in_rv,
                           np.asarray(b_in, np.float32))
    _, sh_in2 = bn_fold(bn_in_g, bn_in_b, bn_in_rm, bn_in_rv)
    sc_emb, sh_emb = bn_fold(bn_emb_g, bn_emb_b, bn_emb_rm, bn_emb_rv,
                             np.asarray(b_emb, np.float32))
    sc_fc, sh_fc = bn_fold(bn_fc_g, bn_fc_b, bn_fc_rm, bn_fc_rv,
                           np.asarray(b_fc, np.float32))

    W_gs = np.asarray(W_gs, np.float32)
    wgs_sT = _f32(np.stack([W_gs[l][:, :NH].T for l in range(L)]))
    wgs_nT = _f32(np.stack([W_gs[l][:, NH:].T for l in range(L)]))
    bgs = _f32(np.asarray(b_gs, np.float32).T)          # [NH, L]

    wihT = np.stack([_bf(np.asarray(Wih0, np.float32).T),
                     _bf(np.asarray(Wih1, np.float32).T)])
    whhT = np.stack([_bf(np.asarray(Whh0, np.float32).T),
                     _bf(np.asarray(Whh1, np.float32).T)])
    bl = np.stack([np.asarray(bih0, np.float32) + np.asarray(bhh0, np.float32),
                   np.asarray(bih1, np.float32) + np.asarray(bhh1, np.float32)])
    # [512] per layer -> [128, l*4+g]
    blstm = np.zeros((NH, 8), np.float32)
    for l in range(2):
        for g in range(4):
            blstm[:, l * 4 + g] = bl[l][g * NH:(g + 1) * NH]

    w_embT = _f32(np.asarray(W_emb, np.float32).T)
    w_fcT = _f32(np.asarray(W_fc, np.float32).T)
    w_outT = _f32(np.asarray(W_out, np.float32).T)
    bout = _f32(np.tile(np.asarray(b_out, np.float32)[None, :], (128, 1)))

    shared = {
        "w_inT": w_inT,
        "sc_in": sc_in[:, None], "sh_in": sh_in[:, None],
        "sc_in_h": _f32(0.5 * sc_in)[:, None], "sh_in2": sh_in2[:, None],
        "wgs_sT": wgs_sT, "wgs_nT": wgs_nT, "bgs": bgs,
        "wihT": _bf(wihT), "whhT": _bf(whhT), "blstm": blstm,
        "w_embT": w_embT, "sc_emb": sc_emb[:, None], "sh_emb": sh_emb[:, None],
        "w_fcT": w_fcT,
        "sc_fc_a": _f32(sc_fc[:128])[:, None], "sh_fc_a": _f32(sh_fc[:128])[:, None],
        "sc_fc_b": _f32(sc_fc[128:])[:, None], "sh_fc_b": _f32(sh_fc[128:])[:, None],
        "w_outT": w_outT, "bout": bout,
    }

    # adjacency: per-core transposed bf16 shard with padded global ordering
    adj_bf = _bf(adj)
    rowsum = adj.sum(axis=1)                     # fp32, exact rows
    in_maps = []
    for c in range(NC):
        rows = slice(c * NPC, (c + 1) * NPC)
        adjT = np.zeros((NP, PC), ml_dtypes.bfloat16)
        blk = adj_bf[rows].T                     # [10000, 1250] view
        for ck in range(NC):
            adjT[ck * PC:ck * PC + NPC, :NPC] = blk[ck * NPC:(ck + 1) * NPC]
        xT = np.zeros((FPAD, PC), ml_dtypes.bfloat16)
        xT[:NFEAT, :NPC] = _bf(x[rows].T)
        embT = np.zeros((NFE, PC), np.float32)
        embT[:, :NPC] = embed[rows].T
        rec = np.zeros((PC,), np.float32)
        rec[:NPC] = 1.0 / rowsum[rows]
        rsb = np.ascontiguousarray(
            np.broadcast_to(rec[None, :], (128, PC)), dtype=np.float32)
        m = {"adjT": adjT, "xT": xT, "embT": embT, "rsb": rsb}
        m.update(shared)
        in_maps.append(m)
    return in_maps


def kernel(**inputs) -> np.ndarray:
    global _CACHED_NC, LAST_RESULT
    in_maps = _stage_inputs(**inputs)
    if _CACHED_NC is None:
        _CACHED_NC = _build_program()
    nc = _CACHED_NC
    trace = bool(int(os.environ.get("GSAGE_TRACE", "0")))
    res = run_bass_kernel_spmd(
        nc, in_maps, core_ids=list(range(NC)), trace=trace,
    )
    LAST_RESULT = res
    out = np.concatenate(
        [res.results[c]["out"][:NPC] for c in range(NC)], axis=0)
    return np.ascontiguousarray(out, np.float32)


if __name__ == "__main__":
    import reference
    inputs = reference.setup_inputs()
    out = kernel(**{k: np.asarray(v) for k, v in inputs.items()})
    print("out", out.shape, out.dtype)
